# revision 8
# baseline (speedup 1.0000x reference)
"""Expert-parallel MoE (top-2 of 8) Trainium2 Bass kernel — fp8 DoubleRow.

Problem: tokens (2,1024,768), 8 experts, SwiGLU FFN (H=3072),
token-choice top-2 routing. Only routed (token, expert) pairs
contribute; core e handles expert e's routed tokens (padded to C).

Strategy: all matmuls in fp8e4m3 with the DoubleRow perf mode
(256-deep contraction per pass, 0.5 cycles/row — 4x the fp32r rate).
fp8's 3-bit mantissa alone is too coarse (rel err ~6e-2 vs the 2e-2
gate), so precision is tiered by combine weight:

  - Tier 3 (columns [0:C3], the tokens with the LARGEST combine
    weights, per expert): every operand is split hi+lo into two fp8
    tensors at a shared scale (a ~= hi + lo, ~9-10 effective mantissa
    bits). Each matmul runs 3 passes (hi@hi + lo@hi + hi@lo), i.e.
    3/4 the cycles of a bf16 matmul at ~bf16 accuracy.
  - Tier 1 (columns [C3:C], small combine weights): single-pass raw
    fp8. Its ~2.7% rms error enters the output scaled by the small
    combine weight, keeping the global metric under the gate.

Per (mh, q) chunk: G/V accumulate in PSUM, then
  t1 = gelu(g_ps * 1/(Sx*Sw))            (Act)
  u32 = (v_ps * Su/(Sx*Sw)) * t1 = Su*U  (DVE scalar_tensor_tensor)
  uh = fp8(u32)                          (DVE copy; tier-1: DVE stt
                                          writes fp8 directly)
  ul = fp8(u32 - uh)                     (GPSIMD tensor_sub, tier 3)
Phase 2 contracts U (hi/lo fp8) against Wo (hi/lo fp8) the same way;
tier-1 output gets the Wo-lo pass too (2-pass out, cheap margin).

Scales are chosen so every quantized tensor peaks at ~70-90% of
e4m3's max (240): Sx=32, Sw=4096, Swo=2^18, Su=16. Host folds
1/(Su*SWO) into the final combine multiply.
"""

import numpy as np
import ml_dtypes

import concourse.bass as bass
import concourse.mybir as mybir
import concourse.tile as tile
from concourse.bacc import Bacc
from concourse.bass import ds
from concourse.bass_utils import run_bass_kernel_spmd

# Problem constants (fixed by the grading harness's input shapes).
B, N, D, E, H = 2, 1024, 768, 8, 3072
T = B * N
P = 128
KD = D // P     # 6 chunks of the model dim
KH = H // P     # 24 chunks of the hidden dim
KDP = KD // 2   # 3 DoubleRow k-pairs over D
KHP = KH // 2   # 12 DoubleRow k-pairs over H
N_CORES = 8

SX = 32.0        # x scale      (absmax ~5.1 -> 162)
SW = 4096.0      # Wg/Wv scale  (xavier bound 0.0395 -> 162)
SWO = 262144.0   # Wo scale     (xavier bound 7.9e-4 -> 207)
SU = 16.0        # U scale      (absmax ~8.6 -> 138)

E4 = ml_dtypes.float8_e4m3

_NC_CACHE: dict = {}
LAST_RESULTS = None  # BassKernelResults of the most recent kernel() call
LAST_NC = None       # the bass.Bass program used for the last call


def _build_nc(
    C3: int,
    C1: int,
    act: str = "Gelu",
    warmup: int = 20,
    w1_bufs: int = 18,
    ps_g_bufs: int = 3,
    ps_v_bufs: int = 2,
    ps_o_bufs: int = 3,
    reps: int = 1,
) -> bass.Bass:
    """One-expert tiered-fp8 FFN over C3+C1 padded tokens; SPMD x8."""
    C = C3 + C1
    assert C3 <= 512 and C1 <= 512, (C3, C1)
    assert ps_g_bufs + ps_v_bufs + ps_o_bufs <= 8
    f32 = mybir.dt.float32
    f8 = mybir.dt.float8e4
    DR = mybir.MatmulPerfMode.DoubleRow
    GELU = getattr(mybir.ActivationFunctionType, act)
    COPY = mybir.ActivationFunctionType.Copy
    MUL = mybir.AluOpType.mult
    gsc = 1.0 / (SX * SW)   # psum -> gelu argument
    cu = SU / (SX * SW)     # psum -> Su*V

    nc = Bacc()
    # x_h[d, kd, c]       = fp8(SX * x_pad[c, kd*128+d]); cols comb-sorted desc
    # x_l[d, kd, c<C3]    = fp8(SX*x - x_h)
    # wg_h[mh, d, j, h]   : j in [0:KD) hi blocks, [KD:2KD) lo blocks
    #                       hi = fp8(SW * Wg[mh*128+h, kd*128+d]), lo = resid
    # wo_h[md, h, j, d]   : j in [0:KH) hi, [KH:2KH) lo, scale SWO
    # out[md, d, c]       = (SU*SWO) * expert_out^T[md*128+d, c]  (fp32)
    x_d = nc.declare_dram_parameter("x", [P, KD, C], f8, isOutput=False)
    xl_d = nc.declare_dram_parameter("xl", [P, KD, C3], f8, isOutput=False)
    wg_d = nc.declare_dram_parameter("wg", [KH, P, 2 * KD, P], f8, isOutput=False)
    wv_d = nc.declare_dram_parameter("wv", [KH, P, 2 * KD, P], f8, isOutput=False)
    wo_d = nc.declare_dram_parameter("wo", [KD, P, 2 * KH, P], f8, isOutput=False)
    out_d = nc.declare_dram_parameter("out", [KD, P, C], f32, isOutput=True)

    with tile.TileContext(nc) as tc:
        with (
            tc.tile_pool(name="singles", bufs=1) as singles,
            tc.tile_pool(name="w1", bufs=w1_bufs) as w1pool,
            tc.tile_pool(name="w2", bufs=1) as w2pool,
            tc.tile_pool(name="tmp", bufs=4) as tmppool,
            tc.tile_pool(name="outp", bufs=4) as outpool,
            tc.tile_pool(name="psg", bufs=ps_g_bufs, space="PSUM") as psg,
            tc.tile_pool(name="psv", bufs=ps_v_bufs, space="PSUM") as psv,
            tc.tile_pool(name="pso", bufs=ps_o_bufs, space="PSUM") as pso,
        ):
            if warmup:
                # PE clock (HAM) warm-up covering the initial DMA window;
                # memset (no DMA dependency) so it starts immediately.
                wz = singles.tile([P, 256], f8)
                nc.vector.memset(wz[:], 0)
                pw = pso.tile([P, 512], f32, name="o_ps", tag="o_ps")
                for _ in range(warmup):
                    nc.tensor.matmul(
                        pw[:, 0:256], wz[:, 0:P], wz[:], start=True, stop=True
                    )

            for _rep in range(reps):
                xs = singles.tile([P, KD, C], f8)
                nc.sync.dma_start(out=xs[:], in_=x_d[:])
                xls = singles.tile([P, KD, C3], f8)
                nc.sync.dma_start(out=xls[:], in_=xl_d[:])
                ut_h = singles.tile([P, KH, C], f8)
                ut_l = singles.tile([P, KH, C3], f8)
                # Phase-2 weights: allocated upfront; half-tile DMAs
                # spread through phase 1 so they never starve the per-mh
                # wg/wv loads (DMA bus is a serialized shared resource).
                wo_ts = [
                    w2pool.tile([P, 2 * KH, P], f8, name=f"wo_t{md}")
                    for md in range(KD)
                ]

                # Phase 1: U^T chunk by chunk over mh.
                for mh in range(KH):
                    wg_t = w1pool.tile([P, 2 * KD, P], f8)
                    nc.sync.dma_start(out=wg_t[:], in_=wg_d[mh])
                    wv_t = w1pool.tile([P, 2 * KD, P], f8)
                    nc.sync.dma_start(out=wv_t[:], in_=wv_d[mh])
                    if mh % 2 == 1:
                        md, half = (mh - 1) // 4, ((mh - 1) // 2) % 2
                        hs = ds(half * KH, KH)
                        nc.sync.dma_start(
                            out=wo_ts[md][:, hs], in_=wo_d[md][:, hs]
                        )

                    # ---- tier-1 chunk first: cols [C3:C], single-pass.
                    # Its short gelu->mul chain means ut_h[:, :, C3:C] is
                    # complete almost immediately after the last phase-1
                    # matmul, letting phase 2's tier-1 chains start with no
                    # barrier bubble. Chunk psums use the pso ring (idle
                    # during phase 1) so tier-3 rotation keeps full depth.
                    g1 = pso.tile([P, 512], f32, name="o_ps", tag="o_ps")
                    v1 = pso.tile([P, 512], f32, name="o_ps", tag="o_ps")
                    for ps_t, w_t in ((g1, wg_t), (v1, wv_t)):
                        for kp in range(KDP):
                            nc.tensor.matmul(
                                ps_t[:, 0:C1],
                                w_t[:, ds(2 * kp, 2)],
                                xs[:, ds(2 * kp, 2), C3:C],
                                start=(kp == 0),
                                stop=(kp == KDP - 1),
                                perf_mode=DR,
                            )
                    t1b = tmppool.tile([P, 512], f32, name="t1", tag="t1")
                    nc.scalar.activation(
                        out=t1b[:, 0:C1], in_=g1[:, 0:C1], func=GELU, scale=gsc
                    )
                    nc.vector.scalar_tensor_tensor(
                        out=ut_h[:, mh, C3:C], in0=v1[:, 0:C1], scalar=cu,
                        in1=t1b[:, 0:C1], op0=MUL, op1=MUL,
                    )

                    # ---- tier-3 chunk: cols [0:C3], 3-pass hi/lo ----
                    g3 = psg.tile([P, 512], f32, name="g_ps", tag="g_ps")
                    v3 = psv.tile([P, 512], f32, name="v_ps", tag="v_ps")
                    for ps_t, w_t in ((g3, wg_t), (v3, wv_t)):
                        i = 0
                        for mov, woff in ((xs, 0), (xs, KD), (xls, 0)):
                            for kp in range(KDP):
                                nc.tensor.matmul(
                                    ps_t[:, 0:C3],
                                    w_t[:, ds(woff + 2 * kp, 2)],
                                    mov[:, ds(2 * kp, 2), 0:C3],
                                    start=(i == 0),
                                    stop=(i == 3 * KDP - 1),
                                    perf_mode=DR,
                                )
                                i += 1
                    t1 = tmppool.tile([P, 512], f32, name="t1", tag="t1")
                    nc.scalar.activation(
                        out=t1[:, 0:C3], in_=g3[:, 0:C3], func=GELU, scale=gsc
                    )
                    u32 = tmppool.tile([P, 512], f32, name="u32", tag="u32")
                    nc.vector.scalar_tensor_tensor(
                        out=u32[:, 0:C3], in0=v3[:, 0:C3], scalar=cu,
                        in1=t1[:, 0:C3], op0=MUL, op1=MUL,
                    )
                    nc.vector.tensor_copy(ut_h[:, mh, 0:C3], u32[:, 0:C3])
                    # Residual on the otherwise-idle GPSIMD engine.
                    nc.gpsimd.tensor_sub(
                        ut_l[:, mh, 0:C3], u32[:, 0:C3], ut_h[:, mh, 0:C3]
                    )

                # Phase 2: out^T[md] = sum_kh Wo-blocks @ U^T[kh].
                # Tier-1 chains first: they only need ut_h[:, :, C3:C]
                # (whose producer chain is short), hiding the tail of the
                # last tier-3 gelu/mul/sub chain behind real PE work. The
                # idle phase-1 psg ring gives them their own rotation.
                # Tier-1 chains for md 0..4 up front; md 5's tier-1
                # chain runs LAST so the kernel tail drains behind the
                # smallest possible copy+DMA (168 cols vs 432).
                for md in list(range(KD - 1)) + [None]:
                    if md is None:
                        break
                    wo_t = wo_ts[md]
                    o1 = psg.tile([P, 512], f32, name="g_ps", tag="g_ps")
                    i = 0
                    for woff in (0, KH):   # Wo hi + lo: tier-1 2-pass out
                        for kp in range(KHP):
                            nc.tensor.matmul(
                                o1[:, 0:C1],
                                wo_t[:, ds(woff + 2 * kp, 2)],
                                ut_h[:, ds(2 * kp, 2), C3:C],
                                start=(i == 0),
                                stop=(i == 2 * KHP - 1),
                                perf_mode=DR,
                            )
                            i += 1
                    o1_t = outpool.tile([P, C1], f32, name="o1_t", tag="o1_t")
                    if md % 2 == 0:
                        nc.vector.tensor_copy(o1_t[:], o1[:, 0:C1])
                    else:
                        nc.scalar.activation(
                            out=o1_t[:], in_=o1[:, 0:C1], func=COPY
                        )
                    nc.sync.dma_start(out=out_d[md, :, C3:C], in_=o1_t[:])

                for md in range(KD):
                    wo_t = wo_ts[md]
                    op = pso if md % 2 == 0 else psv
                    o3 = op.tile(
                        [P, 512], f32,
                        name="o_ps" if md % 2 == 0 else "v_ps",
                        tag="o_ps" if md % 2 == 0 else "v_ps",
                    )
                    i = 0
                    for mov, woff in ((ut_h, 0), (ut_h, KH), (ut_l, 0)):
                        for kp in range(KHP):
                            nc.tensor.matmul(
                                o3[:, 0:C3],
                                wo_t[:, ds(woff + 2 * kp, 2)],
                                mov[:, ds(2 * kp, 2), 0:C3],
                                start=(i == 0),
                                stop=(i == 3 * KHP - 1),
                                perf_mode=DR,
                            )
                            i += 1
                    o3_t = outpool.tile([P, C3], f32, name="o3_t", tag="o3_t")
                    if md % 2 == 0:
                        nc.vector.tensor_copy(o3_t[:], o3[:, 0:C3])
                    else:
                        nc.scalar.activation(
                            out=o3_t[:], in_=o3[:, 0:C3], func=COPY
                        )
                    nc.sync.dma_start(out=out_d[md, :, 0:C3], in_=o3_t[:])

                md = KD - 1
                wo_t = wo_ts[md]
                o1 = psg.tile([P, 512], f32, name="g_ps", tag="g_ps")
                i = 0
                for woff in (0, KH):
                    for kp in range(KHP):
                        nc.tensor.matmul(
                            o1[:, 0:C1],
                            wo_t[:, ds(woff + 2 * kp, 2)],
                            ut_h[:, ds(2 * kp, 2), C3:C],
                            start=(i == 0),
                            stop=(i == 2 * KHP - 1),
                            perf_mode=DR,
                        )
                        i += 1
                o1_t = outpool.tile([P, C1], f32, name="o1_t", tag="o1_t")
                nc.vector.tensor_copy(o1_t[:], o1[:, 0:C1])
                nc.sync.dma_start(out=out_d[md, :, C3:C], in_=o1_t[:])

    nc.finalize()
    return nc


def _get_nc(C3: int, C1: int, act: str = "Gelu", **kw) -> bass.Bass:
    key = (C3, C1, act, tuple(sorted(kw.items())))
    if key not in _NC_CACHE:
        _NC_CACHE[key] = _build_nc(C3, C1, act, **kw)
    return _NC_CACHE[key]


def _capacity(max_cnt: int) -> tuple[int, int]:
    """(C3, C1): C3+C1 >= max_cnt, tier-3 share ~72%, both <= 512."""
    C = max(-(-max_cnt // 4) * 4, 128)
    assert C <= 1024, f"capacity {C} needs more than 2 chunks"
    C3 = min(512, (C * 44 // 75 + 7) // 8 * 8)
    C1 = C - C3
    assert 0 < C1 <= 512
    return C3, C1


def _hilo(a: np.ndarray, s: float) -> tuple[np.ndarray, np.ndarray]:
    """Same-scale fp8 hi/lo split of a*s.

    Clipped to +-224 so out-of-range values saturate (finite) instead
    of rounding to e4m3 inf; the fixed scales leave >1.3x headroom for
    the reference input distribution, so the clip never fires there.
    """
    sa = np.clip(np.asarray(a * s, np.float32), -224.0, 224.0)
    h = sa.astype(E4)
    l = np.clip(sa - h.astype(np.float32), -224.0, 224.0).astype(E4)
    return h, l


def _wlayout(w: np.ndarray, s: float, ko: int, ki: int) -> np.ndarray:
    """(ko*128, ki*128) weight -> [ko, P, 2*ki, P] fp8 hi||lo blocks."""
    h, l = _hilo(w, s)

    def blk(a):
        return a.reshape(ko, P, ki, P).transpose(0, 3, 2, 1)

    return np.ascontiguousarray(np.concatenate([blk(h), blk(l)], axis=2))


def _xlayout(a: np.ndarray, C: int) -> np.ndarray:
    """(cols<=C, D) fp8 -> [P, KD, C] (partition-major blocks)."""
    buf = np.zeros((C, D), E4)
    buf[: len(a)] = a
    return np.ascontiguousarray(
        buf.T.reshape(KD, P, C).transpose(1, 0, 2)
    )


def _prep_in_maps(x, Wg, Wv, Wo, C3, C1, idxs, cnts):
    C = C3 + C1
    in_maps = []
    for e in range(E):
        xg = x[idxs[e]]
        xh, xl = _hilo(xg, SX)
        in_maps.append(
            {
                "x": _xlayout(xh, C),
                "xl": _xlayout(xl[:C3], C3),
                "wg": _wlayout(Wg[e], SW, KH, KD),
                "wv": _wlayout(Wv[e], SW, KH, KD),
                "wo": _wlayout(Wo[e], SWO, KD, KH),
            }
        )
    return in_maps


def kernel(
    tokens, dispatch_weights, combine_weights, Wg, Wv, Wo, scale, **run_kwargs
):
    x = np.ascontiguousarray(np.asarray(tokens, np.float32).reshape(T, D))
    dw = np.asarray(dispatch_weights, np.float32).reshape(T, E)
    cw = np.asarray(combine_weights, np.float32).reshape(T, E)
    Wg = np.ascontiguousarray(np.asarray(Wg, np.float32))
    Wv = np.ascontiguousarray(np.asarray(Wv, np.float32))
    Wo = np.ascontiguousarray(np.asarray(Wo, np.float32))
    scale = np.asarray(scale, np.float32)

    mask = dw > 0
    comb = np.where(mask, cw, 0.0).astype(np.float32)
    # Routed tokens per expert, sorted by combine weight DESC so the
    # first C3 columns (tier 3) are the ones that matter most.
    xn = np.linalg.norm(x, axis=1)
    idxs = []
    for e in range(E):
        ids = np.nonzero(mask[:, e])[0]
        ids = ids[np.argsort(-comb[ids, e] * xn[ids], kind="stable")]
        idxs.append(ids)
    cnts = [len(i) for i in idxs]
    C3, C1 = _capacity(max(cnts))

    nc = _get_nc(C3, C1)
    in_maps = _prep_in_maps(x, Wg, Wv, Wo, C3, C1, idxs, cnts)
    res = run_bass_kernel_spmd(
        nc, in_maps, core_ids=list(range(N_CORES)), **run_kwargs
    )
    global LAST_RESULTS, LAST_NC
    LAST_RESULTS = res
    LAST_NC = nc

    C = C3 + C1
    y = np.zeros((T, D), np.float32)
    for e in range(E):
        outT = np.asarray(res.results[e]["out"]).reshape(D, C)
        w = comb[idxs[e], e] * (scale[e] / (SU * SWO))
        y[idxs[e]] += outT.T[: cnts[e]] * w[:, None]
    return y.reshape(B, N, D)


# revision 9
# speedup vs baseline: 1.0130x; 1.0130x over previous
"""Expert-parallel MoE (top-2 of 8) Trainium2 Bass kernel — fp8 DoubleRow.

Problem: tokens (2,1024,768), 8 experts, SwiGLU FFN (H=3072),
token-choice top-2 routing. Only routed (token, expert) pairs
contribute; core e handles expert e's routed tokens (padded to C).

Strategy: all matmuls in fp8e4m3 with the DoubleRow perf mode
(256-deep contraction per pass, 0.5 cycles/row — 4x the fp32r rate).
fp8's 3-bit mantissa alone is too coarse (rel err ~6e-2 vs the 2e-2
gate), so precision is tiered by combine weight:

  - Tier 3 (columns [0:C3], the tokens with the LARGEST combine
    weights, per expert): every operand is split hi+lo into two fp8
    tensors at a shared scale (a ~= hi + lo, ~9-10 effective mantissa
    bits). Each matmul runs 3 passes (hi@hi + lo@hi + hi@lo), i.e.
    3/4 the cycles of a bf16 matmul at ~bf16 accuracy.
  - Tier 1 (columns [C3:C], small combine weights): single-pass raw
    fp8. Its ~2.7% rms error enters the output scaled by the small
    combine weight, keeping the global metric under the gate.

Per (mh, q) chunk: G/V accumulate in PSUM, then
  t1 = gelu(g_ps * 1/(Sx*Sw))            (Act)
  u32 = (v_ps * Su/(Sx*Sw)) * t1 = Su*U  (DVE scalar_tensor_tensor)
  uh = fp8(u32)                          (DVE copy; tier-1: DVE stt
                                          writes fp8 directly)
  ul = fp8(u32 - uh)                     (GPSIMD tensor_sub, tier 3)
Phase 2 contracts U (hi/lo fp8) against Wo (hi/lo fp8) the same way;
tier-1 output gets the Wo-lo pass too (2-pass out, cheap margin).

Scales are chosen so every quantized tensor peaks at ~70-90% of
e4m3's max (240): Sx=32, Sw=4096, Swo=2^18, Su=16. Host folds
1/(Su*SWO) into the final combine multiply.
"""

import numpy as np
import ml_dtypes

import concourse.bass as bass
import concourse.mybir as mybir
import concourse.tile as tile
from concourse.bacc import Bacc
from concourse.bass import ds
from concourse.bass_utils import run_bass_kernel_spmd

# Problem constants (fixed by the grading harness's input shapes).
B, N, D, E, H = 2, 1024, 768, 8, 3072
T = B * N
P = 128
KD = D // P     # 6 chunks of the model dim
KH = H // P     # 24 chunks of the hidden dim
KDP = KD // 2   # 3 DoubleRow k-pairs over D
KHP = KH // 2   # 12 DoubleRow k-pairs over H
N_CORES = 8

SX = 32.0        # x scale      (absmax ~5.1 -> 162)
SW = 4096.0      # Wg/Wv scale  (xavier bound 0.0395 -> 162)
SWO = 262144.0   # Wo scale     (xavier bound 7.9e-4 -> 207)
SU = 16.0        # U scale      (absmax ~8.6 -> 138)

E4 = ml_dtypes.float8_e4m3

_NC_CACHE: dict = {}
LAST_RESULTS = None  # BassKernelResults of the most recent kernel() call
LAST_NC = None       # the bass.Bass program used for the last call


def _build_nc(
    C3: int,
    C1: int,
    act: str = "Gelu",
    warmup: int = 14,
    w1_bufs: int = 18,
    ps_g_bufs: int = 3,
    ps_v_bufs: int = 2,
    ps_o_bufs: int = 3,
    reps: int = 1,
) -> bass.Bass:
    """One-expert tiered-fp8 FFN over C3+C1 padded tokens; SPMD x8."""
    C = C3 + C1
    assert C3 <= 512 and C1 <= 512, (C3, C1)
    assert ps_g_bufs + ps_v_bufs + ps_o_bufs <= 8
    f32 = mybir.dt.float32
    f8 = mybir.dt.float8e4
    DR = mybir.MatmulPerfMode.DoubleRow
    GELU = getattr(mybir.ActivationFunctionType, act)
    COPY = mybir.ActivationFunctionType.Copy
    MUL = mybir.AluOpType.mult
    gsc = 1.0 / (SX * SW)   # psum -> gelu argument
    cu = SU / (SX * SW)     # psum -> Su*V

    nc = Bacc()
    # x_h[d, kd, c]       = fp8(SX * x_pad[c, kd*128+d]); cols comb-sorted desc
    # x_l[d, kd, c<C3]    = fp8(SX*x - x_h)
    # wg_h[mh, d, j, h]   : j in [0:KD) hi blocks, [KD:2KD) lo blocks
    #                       hi = fp8(SW * Wg[mh*128+h, kd*128+d]), lo = resid
    # wo_h[md, h, j, d]   : j in [0:KH) hi, [KH:2KH) lo, scale SWO
    # out[md, d, c]       = (SU*SWO) * expert_out^T[md*128+d, c]  (fp32)
    x_d = nc.declare_dram_parameter("x", [P, KD, C], f8, isOutput=False)
    xl_d = nc.declare_dram_parameter("xl", [P, KD, C3], f8, isOutput=False)
    wg_d = nc.declare_dram_parameter("wg", [KH, P, 2 * KD, P], f8, isOutput=False)
    wv_d = nc.declare_dram_parameter("wv", [KH, P, 2 * KD, P], f8, isOutput=False)
    wo_d = nc.declare_dram_parameter("wo", [KD, P, 2 * KH, P], f8, isOutput=False)
    out_d = nc.declare_dram_parameter("out", [KD, P, C], f32, isOutput=True)

    with tile.TileContext(nc) as tc:
        with (
            tc.tile_pool(name="singles", bufs=1) as singles,
            tc.tile_pool(name="w1", bufs=w1_bufs) as w1pool,
            tc.tile_pool(name="w2", bufs=1) as w2pool,
            tc.tile_pool(name="tmp", bufs=4) as tmppool,
            tc.tile_pool(name="outp", bufs=4) as outpool,
            tc.tile_pool(name="psg", bufs=ps_g_bufs, space="PSUM") as psg,
            tc.tile_pool(name="psv", bufs=ps_v_bufs, space="PSUM") as psv,
            tc.tile_pool(name="pso", bufs=ps_o_bufs, space="PSUM") as pso,
        ):
            if warmup:
                # PE clock (HAM) warm-up covering the initial DMA window;
                # memset (no DMA dependency) so it starts immediately.
                wz = singles.tile([P, 256], f8)
                nc.vector.memset(wz[:], 0)
                pw = pso.tile([P, 512], f32, name="o_ps", tag="o_ps")
                for _ in range(warmup):
                    nc.tensor.matmul(
                        pw[:, 0:256], wz[:, 0:P], wz[:], start=True, stop=True
                    )

            for _rep in range(reps):
                xs = singles.tile([P, KD, C], f8)
                nc.sync.dma_start(out=xs[:], in_=x_d[:])
                # mh=0 weights before xl: the first chunk's tier-1 work
                # and the first 12 tier-3 matmuls (xh passes) need only
                # xs+wg0+wv0; the xl pass sits last in the psum chain, so
                # xl streams in behind them without stalling the PE.
                wg0_t = w1pool.tile([P, 2 * KD, P], f8, name="wg_t")
                nc.sync.dma_start(out=wg0_t[:], in_=wg_d[0])
                wv0_t = w1pool.tile([P, 2 * KD, P], f8, name="wv_t")
                nc.sync.dma_start(out=wv0_t[:], in_=wv_d[0])
                xls = singles.tile([P, KD, C3], f8)
                nc.sync.dma_start(out=xls[:], in_=xl_d[:])
                ut_h = singles.tile([P, KH, C], f8)
                ut_l = singles.tile([P, KH, C3], f8)
                # Phase-2 weight tiles, allocated upfront (DMAs issued
                # after the phase-1 weight stream below).
                wo_ts = [
                    w2pool.tile([P, 2 * KH, P], f8, name=f"wo_t{md}")
                    for md in range(KD)
                ]

                # Phase 1: U^T chunk by chunk over mh.
                for mh in range(KH):
                    if mh == 0:
                        wg_t, wv_t = wg0_t, wv0_t
                    else:
                        wg_t = w1pool.tile([P, 2 * KD, P], f8, name="wg_t")
                        nc.sync.dma_start(out=wg_t[:], in_=wg_d[mh])
                        wv_t = w1pool.tile([P, 2 * KD, P], f8, name="wv_t")
                        nc.sync.dma_start(out=wv_t[:], in_=wv_d[mh])

                    # ---- tier-1 chunk first: cols [C3:C], single-pass.
                    # Its short gelu->mul chain means ut_h[:, :, C3:C] is
                    # complete almost immediately after the last phase-1
                    # matmul, letting phase 2's tier-1 chains start with no
                    # barrier bubble. Chunk psums use the pso ring (idle
                    # during phase 1) so tier-3 rotation keeps full depth.
                    g1 = pso.tile([P, 512], f32, name="o_ps", tag="o_ps")
                    v1 = pso.tile([P, 512], f32, name="o_ps", tag="o_ps")
                    for ps_t, w_t in ((g1, wg_t), (v1, wv_t)):
                        for kp in range(KDP):
                            nc.tensor.matmul(
                                ps_t[:, 0:C1],
                                w_t[:, ds(2 * kp, 2)],
                                xs[:, ds(2 * kp, 2), C3:C],
                                start=(kp == 0),
                                stop=(kp == KDP - 1),
                                perf_mode=DR,
                            )
                    t1b = tmppool.tile([P, 512], f32, name="t1", tag="t1")
                    nc.scalar.activation(
                        out=t1b[:, 0:C1], in_=g1[:, 0:C1], func=GELU, scale=gsc
                    )
                    nc.vector.scalar_tensor_tensor(
                        out=ut_h[:, mh, C3:C], in0=v1[:, 0:C1], scalar=cu,
                        in1=t1b[:, 0:C1], op0=MUL, op1=MUL,
                    )

                    # ---- tier-3 chunk: cols [0:C3], 3-pass hi/lo ----
                    g3 = psg.tile([P, 512], f32, name="g_ps", tag="g_ps")
                    v3 = psv.tile([P, 512], f32, name="v_ps", tag="v_ps")
                    for ps_t, w_t in ((g3, wg_t), (v3, wv_t)):
                        i = 0
                        for mov, woff in ((xs, 0), (xs, KD), (xls, 0)):
                            for kp in range(KDP):
                                nc.tensor.matmul(
                                    ps_t[:, 0:C3],
                                    w_t[:, ds(woff + 2 * kp, 2)],
                                    mov[:, ds(2 * kp, 2), 0:C3],
                                    start=(i == 0),
                                    stop=(i == 3 * KDP - 1),
                                    perf_mode=DR,
                                )
                                i += 1
                    t1 = tmppool.tile([P, 512], f32, name="t1", tag="t1")
                    nc.scalar.activation(
                        out=t1[:, 0:C3], in_=g3[:, 0:C3], func=GELU, scale=gsc
                    )
                    u32 = tmppool.tile([P, 512], f32, name="u32", tag="u32")
                    nc.vector.scalar_tensor_tensor(
                        out=u32[:, 0:C3], in0=v3[:, 0:C3], scalar=cu,
                        in1=t1[:, 0:C3], op0=MUL, op1=MUL,
                    )
                    nc.vector.tensor_copy(ut_h[:, mh, 0:C3], u32[:, 0:C3])
                    # Residual on the otherwise-idle GPSIMD engine.
                    nc.gpsimd.tensor_sub(
                        ut_l[:, mh, 0:C3], u32[:, 0:C3], ut_h[:, mh, 0:C3]
                    )

                # Phase-2 weights: issued after the whole wg/wv stream
                # (phase 1 is weight-stream-bound at the margin, so wo
                # loads interleaved there stall the PE; here they fill
                # the otherwise-idle DMA window while PE finishes the
                # last phase-1 chunks).
                for md in range(KD):
                    nc.sync.dma_start(out=wo_ts[md][:], in_=wo_d[md])

                # Phase 2: out^T[md] = sum_kh Wo-blocks @ U^T[kh].
                # Tier-1 chains first: they only need ut_h[:, :, C3:C]
                # (whose producer chain is short), hiding the tail of the
                # last tier-3 gelu/mul/sub chain behind real PE work. The
                # idle phase-1 psg ring gives them their own rotation.
                # Tier-1 chains for md 0..4 up front; md 5's tier-1
                # chain runs LAST so the kernel tail drains behind the
                # smallest possible copy+DMA (168 cols vs 432).
                for md in list(range(KD - 1)) + [None]:
                    if md is None:
                        break
                    wo_t = wo_ts[md]
                    o1 = psg.tile([P, 512], f32, name="g_ps", tag="g_ps")
                    i = 0
                    for woff in (0, KH):   # Wo hi + lo: tier-1 2-pass out
                        for kp in range(KHP):
                            nc.tensor.matmul(
                                o1[:, 0:C1],
                                wo_t[:, ds(woff + 2 * kp, 2)],
                                ut_h[:, ds(2 * kp, 2), C3:C],
                                start=(i == 0),
                                stop=(i == 2 * KHP - 1),
                                perf_mode=DR,
                            )
                            i += 1
                    o1_t = outpool.tile([P, C1], f32, name="o1_t", tag="o1_t")
                    if md % 2 == 0:
                        nc.vector.tensor_copy(o1_t[:], o1[:, 0:C1])
                    else:
                        nc.scalar.activation(
                            out=o1_t[:], in_=o1[:, 0:C1], func=COPY
                        )
                    nc.sync.dma_start(out=out_d[md, :, C3:C], in_=o1_t[:])

                for md in range(KD):
                    wo_t = wo_ts[md]
                    op = pso if md % 2 == 0 else psv
                    o3 = op.tile(
                        [P, 512], f32,
                        name="o_ps" if md % 2 == 0 else "v_ps",
                        tag="o_ps" if md % 2 == 0 else "v_ps",
                    )
                    i = 0
                    for mov, woff in ((ut_h, 0), (ut_h, KH), (ut_l, 0)):
                        for kp in range(KHP):
                            nc.tensor.matmul(
                                o3[:, 0:C3],
                                wo_t[:, ds(woff + 2 * kp, 2)],
                                mov[:, ds(2 * kp, 2), 0:C3],
                                start=(i == 0),
                                stop=(i == 3 * KHP - 1),
                                perf_mode=DR,
                            )
                            i += 1
                    o3_t = outpool.tile([P, C3], f32, name="o3_t", tag="o3_t")
                    if md % 2 == 0:
                        nc.vector.tensor_copy(o3_t[:], o3[:, 0:C3])
                    else:
                        nc.scalar.activation(
                            out=o3_t[:], in_=o3[:, 0:C3], func=COPY
                        )
                    nc.sync.dma_start(out=out_d[md, :, 0:C3], in_=o3_t[:])

                md = KD - 1
                wo_t = wo_ts[md]
                o1 = psg.tile([P, 512], f32, name="g_ps", tag="g_ps")
                i = 0
                for woff in (0, KH):
                    for kp in range(KHP):
                        nc.tensor.matmul(
                            o1[:, 0:C1],
                            wo_t[:, ds(woff + 2 * kp, 2)],
                            ut_h[:, ds(2 * kp, 2), C3:C],
                            start=(i == 0),
                            stop=(i == 2 * KHP - 1),
                            perf_mode=DR,
                        )
                        i += 1
                o1_t = outpool.tile([P, C1], f32, name="o1_t", tag="o1_t")
                nc.vector.tensor_copy(o1_t[:], o1[:, 0:C1])
                nc.sync.dma_start(out=out_d[md, :, C3:C], in_=o1_t[:])

    nc.finalize()
    return nc


def _get_nc(C3: int, C1: int, act: str = "Gelu", **kw) -> bass.Bass:
    key = (C3, C1, act, tuple(sorted(kw.items())))
    if key not in _NC_CACHE:
        _NC_CACHE[key] = _build_nc(C3, C1, act, **kw)
    return _NC_CACHE[key]


def _capacity(max_cnt: int) -> tuple[int, int]:
    """(C3, C1): C3+C1 >= max_cnt, tier-3 share ~72%, both <= 512."""
    C = max(-(-max_cnt // 4) * 4, 128)
    assert C <= 1024, f"capacity {C} needs more than 2 chunks"
    C3 = min(512, (C * 44 // 75 + 7) // 8 * 8)
    C1 = C - C3
    assert 0 < C1 <= 512
    return C3, C1


def _hilo(a: np.ndarray, s: float) -> tuple[np.ndarray, np.ndarray]:
    """Same-scale fp8 hi/lo split of a*s.

    Clipped to +-224 so out-of-range values saturate (finite) instead
    of rounding to e4m3 inf; the fixed scales leave >1.3x headroom for
    the reference input distribution, so the clip never fires there.
    """
    sa = np.clip(np.asarray(a * s, np.float32), -224.0, 224.0)
    h = sa.astype(E4)
    l = np.clip(sa - h.astype(np.float32), -224.0, 224.0).astype(E4)
    return h, l


def _wlayout(w: np.ndarray, s: float, ko: int, ki: int) -> np.ndarray:
    """(ko*128, ki*128) weight -> [ko, P, 2*ki, P] fp8 hi||lo blocks."""
    h, l = _hilo(w, s)

    def blk(a):
        return a.reshape(ko, P, ki, P).transpose(0, 3, 2, 1)

    return np.ascontiguousarray(np.concatenate([blk(h), blk(l)], axis=2))


def _xlayout(a: np.ndarray, C: int) -> np.ndarray:
    """(cols<=C, D) fp8 -> [P, KD, C] (partition-major blocks)."""
    buf = np.zeros((C, D), E4)
    buf[: len(a)] = a
    return np.ascontiguousarray(
        buf.T.reshape(KD, P, C).transpose(1, 0, 2)
    )


def _prep_in_maps(x, Wg, Wv, Wo, C3, C1, idxs, cnts):
    C = C3 + C1
    in_maps = []
    for e in range(E):
        xg = x[idxs[e]]
        xh, xl = _hilo(xg, SX)
        in_maps.append(
            {
                "x": _xlayout(xh, C),
                "xl": _xlayout(xl[:C3], C3),
                "wg": _wlayout(Wg[e], SW, KH, KD),
                "wv": _wlayout(Wv[e], SW, KH, KD),
                "wo": _wlayout(Wo[e], SWO, KD, KH),
            }
        )
    return in_maps


def kernel(
    tokens, dispatch_weights, combine_weights, Wg, Wv, Wo, scale, **run_kwargs
):
    x = np.ascontiguousarray(np.asarray(tokens, np.float32).reshape(T, D))
    dw = np.asarray(dispatch_weights, np.float32).reshape(T, E)
    cw = np.asarray(combine_weights, np.float32).reshape(T, E)
    Wg = np.ascontiguousarray(np.asarray(Wg, np.float32))
    Wv = np.ascontiguousarray(np.asarray(Wv, np.float32))
    Wo = np.ascontiguousarray(np.asarray(Wo, np.float32))
    scale = np.asarray(scale, np.float32)

    mask = dw > 0
    comb = np.where(mask, cw, 0.0).astype(np.float32)
    # Routed tokens per expert, sorted by combine weight DESC so the
    # first C3 columns (tier 3) are the ones that matter most.
    xn = np.linalg.norm(x, axis=1)
    idxs = []
    for e in range(E):
        ids = np.nonzero(mask[:, e])[0]
        ids = ids[np.argsort(-comb[ids, e] * xn[ids], kind="stable")]
        idxs.append(ids)
    cnts = [len(i) for i in idxs]
    C3, C1 = _capacity(max(cnts))

    nc = _get_nc(C3, C1)
    in_maps = _prep_in_maps(x, Wg, Wv, Wo, C3, C1, idxs, cnts)
    res = run_bass_kernel_spmd(
        nc, in_maps, core_ids=list(range(N_CORES)), **run_kwargs
    )
    global LAST_RESULTS, LAST_NC
    LAST_RESULTS = res
    LAST_NC = nc

    C = C3 + C1
    y = np.zeros((T, D), np.float32)
    for e in range(E):
        outT = np.asarray(res.results[e]["out"]).reshape(D, C)
        w = comb[idxs[e], e] * (scale[e] / (SU * SWO))
        y[idxs[e]] += outT.T[: cnts[e]] * w[:, None]
    return y.reshape(B, N, D)


# revision 10
# speedup vs baseline: 1.0144x; 1.0014x over previous
"""Expert-parallel MoE (top-2 of 8) Trainium2 Bass kernel — fp8 DoubleRow.

Problem: tokens (2,1024,768), 8 experts, SwiGLU FFN (H=3072),
token-choice top-2 routing. Only routed (token, expert) pairs
contribute; core e handles expert e's routed tokens (padded to C).

Strategy: all matmuls in fp8e4m3 with the DoubleRow perf mode
(256-deep contraction per pass, 0.5 cycles/row — 4x the fp32r rate).
fp8's 3-bit mantissa alone is too coarse (rel err ~6e-2 vs the 2e-2
gate), so precision is tiered by combine weight:

  - Tier 3 (columns [0:C3], the tokens with the LARGEST combine
    weights, per expert): every operand is split hi+lo into two fp8
    tensors at a shared scale (a ~= hi + lo, ~9-10 effective mantissa
    bits). Each matmul runs 3 passes (hi@hi + lo@hi + hi@lo), i.e.
    3/4 the cycles of a bf16 matmul at ~bf16 accuracy.
  - Tier 1 (columns [C3:C], small combine weights): single-pass raw
    fp8. Its ~2.7% rms error enters the output scaled by the small
    combine weight, keeping the global metric under the gate.

Per (mh, q) chunk: G/V accumulate in PSUM, then
  t1 = gelu(g_ps * 1/(Sx*Sw))            (Act)
  u32 = (v_ps * Su/(Sx*Sw)) * t1 = Su*U  (DVE scalar_tensor_tensor)
  uh = fp8(u32)                          (DVE copy; tier-1: DVE stt
                                          writes fp8 directly)
  ul = fp8(u32 - uh)                     (GPSIMD tensor_sub, tier 3)
Phase 2 contracts U (hi/lo fp8) against Wo (hi/lo fp8) the same way;
tier-1 output gets the Wo-lo pass too (2-pass out, cheap margin).

Scales are chosen so every quantized tensor peaks at ~70-90% of
e4m3's max (240): Sx=32, Sw=4096, Swo=2^18, Su=16. Host folds
1/(Su*SWO) into the final combine multiply.
"""

import numpy as np
import ml_dtypes

import concourse.bass as bass
import concourse.mybir as mybir
import concourse.tile as tile
from concourse.bacc import Bacc
from concourse.bass import ds
from concourse.bass_utils import run_bass_kernel_spmd

# Problem constants (fixed by the grading harness's input shapes).
B, N, D, E, H = 2, 1024, 768, 8, 3072
T = B * N
P = 128
KD = D // P     # 6 chunks of the model dim
KH = H // P     # 24 chunks of the hidden dim
KDP = KD // 2   # 3 DoubleRow k-pairs over D
KHP = KH // 2   # 12 DoubleRow k-pairs over H
N_CORES = 8

SX = 32.0        # x scale      (absmax ~5.1 -> 162)
SW = 4096.0      # Wg/Wv scale  (xavier bound 0.0395 -> 162)
SWO = 262144.0   # Wo scale     (xavier bound 7.9e-4 -> 207)
SU = 16.0        # U scale      (absmax ~8.6 -> 138)

E4 = ml_dtypes.float8_e4m3

_NC_CACHE: dict = {}
LAST_RESULTS = None  # BassKernelResults of the most recent kernel() call
LAST_NC = None       # the bass.Bass program used for the last call


def _build_nc(
    C3: int,
    C1: int,
    act: str = "Gelu",
    warmup: int = 14,
    w1_bufs: int = 18,
    ps_g_bufs: int = 3,
    ps_v_bufs: int = 2,
    ps_o_bufs: int = 3,
    reps: int = 1,
) -> bass.Bass:
    """One-expert tiered-fp8 FFN over C3+C1 padded tokens; SPMD x8."""
    C = C3 + C1
    assert C3 <= 512 and C1 <= 512, (C3, C1)
    assert ps_g_bufs + ps_v_bufs + ps_o_bufs <= 8
    f32 = mybir.dt.float32
    f8 = mybir.dt.float8e4
    DR = mybir.MatmulPerfMode.DoubleRow
    GELU = getattr(mybir.ActivationFunctionType, act)
    COPY = mybir.ActivationFunctionType.Copy
    MUL = mybir.AluOpType.mult
    gsc = 1.0 / (SX * SW)   # psum -> gelu argument
    cu = SU / (SX * SW)     # psum -> Su*V

    nc = Bacc()
    # x_h[d, kd, c]       = fp8(SX * x_pad[c, kd*128+d]); cols comb-sorted desc
    # x_l[d, kd, c<C3]    = fp8(SX*x - x_h)
    # wg_h[mh, d, j, h]   : j in [0:KD) hi blocks, [KD:2KD) lo blocks
    #                       hi = fp8(SW * Wg[mh*128+h, kd*128+d]), lo = resid
    # wo_h[md, h, j, d]   : j in [0:KH) hi, [KH:2KH) lo, scale SWO
    # out[md, d, c]       = (SU*SWO) * expert_out^T[md*128+d, c]  (fp32)
    x_d = nc.declare_dram_parameter("x", [P, KD, C], f8, isOutput=False)
    xl_d = nc.declare_dram_parameter("xl", [P, KD, C3], f8, isOutput=False)
    wg_d = nc.declare_dram_parameter("wg", [KH, P, 2 * KD, P], f8, isOutput=False)
    wv_d = nc.declare_dram_parameter("wv", [KH, P, 2 * KD, P], f8, isOutput=False)
    wo_d = nc.declare_dram_parameter("wo", [KD, P, 2 * KH, P], f8, isOutput=False)
    out_d = nc.declare_dram_parameter("out", [KD, P, C], f32, isOutput=True)

    with tile.TileContext(nc) as tc:
        with (
            tc.tile_pool(name="singles", bufs=1) as singles,
            tc.tile_pool(name="w1", bufs=w1_bufs) as w1pool,
            tc.tile_pool(name="w2", bufs=1) as w2pool,
            tc.tile_pool(name="tmp", bufs=4) as tmppool,
            tc.tile_pool(name="outp", bufs=6) as outpool,
            tc.tile_pool(name="psg", bufs=ps_g_bufs, space="PSUM") as psg,
            tc.tile_pool(name="psv", bufs=ps_v_bufs, space="PSUM") as psv,
            tc.tile_pool(name="pso", bufs=ps_o_bufs, space="PSUM") as pso,
        ):
            if warmup:
                # PE clock (HAM) warm-up covering the initial DMA window;
                # memset (no DMA dependency) so it starts immediately.
                wz = singles.tile([P, 256], f8)
                nc.vector.memset(wz[:], 0)
                pw = pso.tile([P, 512], f32, name="o_ps", tag="o_ps")
                for _ in range(warmup):
                    nc.tensor.matmul(
                        pw[:, 0:256], wz[:, 0:P], wz[:], start=True, stop=True
                    )

            for _rep in range(reps):
                xs = singles.tile([P, KD, C], f8)
                nc.sync.dma_start(out=xs[:], in_=x_d[:])
                # mh=0 weights before xl: the first chunk's tier-1 work
                # and the first 12 tier-3 matmuls (xh passes) need only
                # xs+wg0+wv0; the xl pass sits last in the psum chain, so
                # xl streams in behind them without stalling the PE.
                wg0_t = w1pool.tile([P, 2 * KD, P], f8, name="wg_t")
                nc.sync.dma_start(out=wg0_t[:], in_=wg_d[0])
                wv0_t = w1pool.tile([P, 2 * KD, P], f8, name="wv_t")
                nc.sync.dma_start(out=wv0_t[:], in_=wv_d[0])
                xls = singles.tile([P, KD, C3], f8)
                nc.sync.dma_start(out=xls[:], in_=xl_d[:])
                ut_h = singles.tile([P, KH, C], f8)
                ut_l = singles.tile([P, KH, C3], f8)
                # Phase-2 weight tiles, allocated upfront (DMAs issued
                # after the phase-1 weight stream below).
                wo_ts = [
                    w2pool.tile([P, 2 * KH, P], f8, name=f"wo_t{md}")
                    for md in range(KD)
                ]

                # Phase 1: U^T chunk by chunk over mh.
                for mh in range(KH):
                    if mh == 0:
                        wg_t, wv_t = wg0_t, wv0_t
                    else:
                        wg_t = w1pool.tile([P, 2 * KD, P], f8, name="wg_t")
                        nc.sync.dma_start(out=wg_t[:], in_=wg_d[mh])
                        wv_t = w1pool.tile([P, 2 * KD, P], f8, name="wv_t")
                        nc.sync.dma_start(out=wv_t[:], in_=wv_d[mh])

                    # ---- tier-1 chunk first: cols [C3:C], single-pass.
                    # Its short gelu->mul chain means ut_h[:, :, C3:C] is
                    # complete almost immediately after the last phase-1
                    # matmul, letting phase 2's tier-1 chains start with no
                    # barrier bubble. Chunk psums use the pso ring (idle
                    # during phase 1) so tier-3 rotation keeps full depth.
                    g1 = pso.tile([P, 512], f32, name="o_ps", tag="o_ps")
                    v1 = pso.tile([P, 512], f32, name="o_ps", tag="o_ps")
                    for ps_t, w_t in ((g1, wg_t), (v1, wv_t)):
                        for kp in range(KDP):
                            nc.tensor.matmul(
                                ps_t[:, 0:C1],
                                w_t[:, ds(2 * kp, 2)],
                                xs[:, ds(2 * kp, 2), C3:C],
                                start=(kp == 0),
                                stop=(kp == KDP - 1),
                                perf_mode=DR,
                            )
                    t1b = tmppool.tile([P, 512], f32, name="t1", tag="t1")
                    nc.scalar.activation(
                        out=t1b[:, 0:C1], in_=g1[:, 0:C1], func=GELU, scale=gsc
                    )
                    nc.vector.scalar_tensor_tensor(
                        out=ut_h[:, mh, C3:C], in0=v1[:, 0:C1], scalar=cu,
                        in1=t1b[:, 0:C1], op0=MUL, op1=MUL,
                    )

                    # ---- tier-3 chunk: cols [0:C3], 3-pass hi/lo ----
                    g3 = psg.tile([P, 512], f32, name="g_ps", tag="g_ps")
                    v3 = psv.tile([P, 512], f32, name="v_ps", tag="v_ps")
                    for ps_t, w_t in ((g3, wg_t), (v3, wv_t)):
                        i = 0
                        for mov, woff in ((xs, 0), (xs, KD), (xls, 0)):
                            for kp in range(KDP):
                                nc.tensor.matmul(
                                    ps_t[:, 0:C3],
                                    w_t[:, ds(woff + 2 * kp, 2)],
                                    mov[:, ds(2 * kp, 2), 0:C3],
                                    start=(i == 0),
                                    stop=(i == 3 * KDP - 1),
                                    perf_mode=DR,
                                )
                                i += 1
                    t1 = tmppool.tile([P, 512], f32, name="t1", tag="t1")
                    nc.scalar.activation(
                        out=t1[:, 0:C3], in_=g3[:, 0:C3], func=GELU, scale=gsc
                    )
                    u32 = tmppool.tile([P, 512], f32, name="u32", tag="u32")
                    nc.vector.scalar_tensor_tensor(
                        out=u32[:, 0:C3], in0=v3[:, 0:C3], scalar=cu,
                        in1=t1[:, 0:C3], op0=MUL, op1=MUL,
                    )
                    nc.vector.tensor_copy(ut_h[:, mh, 0:C3], u32[:, 0:C3])
                    # Residual on the otherwise-idle GPSIMD engine.
                    nc.gpsimd.tensor_sub(
                        ut_l[:, mh, 0:C3], u32[:, 0:C3], ut_h[:, mh, 0:C3]
                    )

                # Phase-2 weights: issued after the whole wg/wv stream
                # (phase 1 is weight-stream-bound at the margin, so wo
                # loads interleaved there stall the PE; here they fill
                # the otherwise-idle DMA window while PE finishes the
                # last phase-1 chunks).
                for md in range(KD):
                    nc.sync.dma_start(out=wo_ts[md][:], in_=wo_d[md])

                # Phase 2: out^T[md] = sum_kh Wo-blocks @ U^T[kh].
                # Tier-1 chains first: they only need ut_h[:, :, C3:C]
                # (whose producer chain is short), hiding the tail of the
                # last tier-3 gelu/mul/sub chain behind real PE work. The
                # idle phase-1 psg ring gives them their own rotation.
                # Tier-1 chains for md 0..4 up front; md 5's tier-1
                # chain runs LAST so the kernel tail drains behind the
                # smallest possible copy+DMA (168 cols vs 432).
                for md in list(range(KD - 1)) + [None]:
                    if md is None:
                        break
                    wo_t = wo_ts[md]
                    o1 = psg.tile([P, 512], f32, name="g_ps", tag="g_ps")
                    i = 0
                    for woff in (0, KH):   # Wo hi + lo: tier-1 2-pass out
                        for kp in range(KHP):
                            nc.tensor.matmul(
                                o1[:, 0:C1],
                                wo_t[:, ds(woff + 2 * kp, 2)],
                                ut_h[:, ds(2 * kp, 2), C3:C],
                                start=(i == 0),
                                stop=(i == 2 * KHP - 1),
                                perf_mode=DR,
                            )
                            i += 1
                    o1_t = outpool.tile([P, C1], f32, name="o1_t", tag="o1_t")
                    if md % 2 == 0:
                        nc.vector.tensor_copy(o1_t[:], o1[:, 0:C1])
                    else:
                        nc.scalar.activation(
                            out=o1_t[:], in_=o1[:, 0:C1], func=COPY
                        )
                    nc.sync.dma_start(out=out_d[md, :, C3:C], in_=o1_t[:])

                for md in range(KD):
                    wo_t = wo_ts[md]
                    op = pso if md % 2 == 0 else psv
                    o3 = op.tile(
                        [P, 512], f32,
                        name="o_ps" if md % 2 == 0 else "v_ps",
                        tag="o_ps" if md % 2 == 0 else "v_ps",
                    )
                    i = 0
                    for mov, woff in ((ut_h, 0), (ut_h, KH), (ut_l, 0)):
                        for kp in range(KHP):
                            nc.tensor.matmul(
                                o3[:, 0:C3],
                                wo_t[:, ds(woff + 2 * kp, 2)],
                                mov[:, ds(2 * kp, 2), 0:C3],
                                start=(i == 0),
                                stop=(i == 3 * KHP - 1),
                                perf_mode=DR,
                            )
                            i += 1
                    o3_t = outpool.tile([P, C3], f32, name="o3_t", tag="o3_t")
                    if md % 2 == 0:
                        nc.vector.tensor_copy(o3_t[:], o3[:, 0:C3])
                    else:
                        nc.scalar.activation(
                            out=o3_t[:], in_=o3[:, 0:C3], func=COPY
                        )
                    nc.sync.dma_start(out=out_d[md, :, 0:C3], in_=o3_t[:])

                md = KD - 1
                wo_t = wo_ts[md]
                o1 = psg.tile([P, 512], f32, name="g_ps", tag="g_ps")
                i = 0
                for woff in (0, KH):
                    for kp in range(KHP):
                        nc.tensor.matmul(
                            o1[:, 0:C1],
                            wo_t[:, ds(woff + 2 * kp, 2)],
                            ut_h[:, ds(2 * kp, 2), C3:C],
                            start=(i == 0),
                            stop=(i == 2 * KHP - 1),
                            perf_mode=DR,
                        )
                        i += 1
                o1_t = outpool.tile([P, C1], f32, name="o1_t", tag="o1_t")
                nc.vector.tensor_copy(o1_t[:], o1[:, 0:C1])
                nc.sync.dma_start(out=out_d[md, :, C3:C], in_=o1_t[:])

    nc.finalize()
    return nc


def _get_nc(C3: int, C1: int, act: str = "Gelu", **kw) -> bass.Bass:
    key = (C3, C1, act, tuple(sorted(kw.items())))
    if key not in _NC_CACHE:
        _NC_CACHE[key] = _build_nc(C3, C1, act, **kw)
    return _NC_CACHE[key]


def _capacity(max_cnt: int) -> tuple[int, int]:
    """(C3, C1): C3+C1 >= max_cnt, tier-3 share ~72%, both <= 512."""
    C = max(-(-max_cnt // 4) * 4, 128)
    assert C <= 1024, f"capacity {C} needs more than 2 chunks"
    C3 = min(512, (C * 44 // 75 + 7) // 8 * 8)
    C1 = C - C3
    assert 0 < C1 <= 512
    return C3, C1


def _hilo(a: np.ndarray, s: float) -> tuple[np.ndarray, np.ndarray]:
    """Same-scale fp8 hi/lo split of a*s.

    Clipped to +-224 so out-of-range values saturate (finite) instead
    of rounding to e4m3 inf; the fixed scales leave >1.3x headroom for
    the reference input distribution, so the clip never fires there.
    """
    sa = np.clip(np.asarray(a * s, np.float32), -224.0, 224.0)
    h = sa.astype(E4)
    l = np.clip(sa - h.astype(np.float32), -224.0, 224.0).astype(E4)
    return h, l


def _wlayout(w: np.ndarray, s: float, ko: int, ki: int) -> np.ndarray:
    """(ko*128, ki*128) weight -> [ko, P, 2*ki, P] fp8 hi||lo blocks."""
    h, l = _hilo(w, s)

    def blk(a):
        return a.reshape(ko, P, ki, P).transpose(0, 3, 2, 1)

    return np.ascontiguousarray(np.concatenate([blk(h), blk(l)], axis=2))


def _xlayout(a: np.ndarray, C: int) -> np.ndarray:
    """(cols<=C, D) fp8 -> [P, KD, C] (partition-major blocks)."""
    buf = np.zeros((C, D), E4)
    buf[: len(a)] = a
    return np.ascontiguousarray(
        buf.T.reshape(KD, P, C).transpose(1, 0, 2)
    )


def _prep_in_maps(x, Wg, Wv, Wo, C3, C1, idxs, cnts):
    C = C3 + C1
    in_maps = []
    for e in range(E):
        xg = x[idxs[e]]
        xh, xl = _hilo(xg, SX)
        in_maps.append(
            {
                "x": _xlayout(xh, C),
                "xl": _xlayout(xl[:C3], C3),
                "wg": _wlayout(Wg[e], SW, KH, KD),
                "wv": _wlayout(Wv[e], SW, KH, KD),
                "wo": _wlayout(Wo[e], SWO, KD, KH),
            }
        )
    return in_maps


def kernel(
    tokens, dispatch_weights, combine_weights, Wg, Wv, Wo, scale, **run_kwargs
):
    x = np.ascontiguousarray(np.asarray(tokens, np.float32).reshape(T, D))
    dw = np.asarray(dispatch_weights, np.float32).reshape(T, E)
    cw = np.asarray(combine_weights, np.float32).reshape(T, E)
    Wg = np.ascontiguousarray(np.asarray(Wg, np.float32))
    Wv = np.ascontiguousarray(np.asarray(Wv, np.float32))
    Wo = np.ascontiguousarray(np.asarray(Wo, np.float32))
    scale = np.asarray(scale, np.float32)

    mask = dw > 0
    comb = np.where(mask, cw, 0.0).astype(np.float32)
    # Routed tokens per expert, sorted by combine weight DESC so the
    # first C3 columns (tier 3) are the ones that matter most.
    xn = np.linalg.norm(x, axis=1)
    idxs = []
    for e in range(E):
        ids = np.nonzero(mask[:, e])[0]
        ids = ids[np.argsort(-comb[ids, e] * xn[ids], kind="stable")]
        idxs.append(ids)
    cnts = [len(i) for i in idxs]
    C3, C1 = _capacity(max(cnts))

    nc = _get_nc(C3, C1)
    in_maps = _prep_in_maps(x, Wg, Wv, Wo, C3, C1, idxs, cnts)
    res = run_bass_kernel_spmd(
        nc, in_maps, core_ids=list(range(N_CORES)), **run_kwargs
    )
    global LAST_RESULTS, LAST_NC
    LAST_RESULTS = res
    LAST_NC = nc

    C = C3 + C1
    y = np.zeros((T, D), np.float32)
    for e in range(E):
        outT = np.asarray(res.results[e]["out"]).reshape(D, C)
        w = comb[idxs[e], e] * (scale[e] / (SU * SWO))
        y[idxs[e]] += outT.T[: cnts[e]] * w[:, None]
    return y.reshape(B, N, D)
